# revision 29
# baseline (speedup 1.0000x reference)
"""Trainium2 Bass kernel for ChebGraphConv forward.

Reference math:
    d = diagonal(Tks, axis1=1, axis2=2)                  # [K, N]
    out = einsum('kn,btnc,kco->btno', d, x, Theta) + sum_k bias[k]

Reformulation: per-node weight W_n = sum_k d[k,n] * Theta[k]  (64x64),
then out[bt, n, :] = x[bt, n, :] @ W_n + bias_sum.

The kernel is HBM/DMA-bound (x in + out dominate). Precision plan within
the 2e-2 rel-err budget: x is quantized host-side to fp8 e3m4 (stored as
2x to keep small values out of the subnormal range; W carries the /2),
W is bf16 (compact, expanded to block-diag on-chip), PSUM accumulates
f32, output is written bf16 and upcast on host. End-to-end rel-err
~1.35e-2 (x-quantization dominates).

DMA on this part is element-rate-limited (~85G elem/s per DGE ring,
byte width free), so all bulk transfers ride as packed f32 words:
x fp8 bytes 4-per-word (bitcast back to fp8 for the matmul rhs), out
bf16 2-per-word. Transfers are split across the three rings (SP /
Activation HWDGE + gpsimd SWDGE) to stay under per-ring caps.

Distribution: shard the N=1024 nodes over 8 cores (128 nodes each); every
core sees all BT=768 (batch*time) rows. Host pre-permutes x to
[p=(pair_half, c), pair, bt] so every DMA is a plain 2D transfer with
KB-scale contiguous runs per partition.

Per group of GRP node pairs:
  - one [128, GRP*768/4] f32 DMA in (fp8 bytes packed)
  - per pair: 2 matmuls (bt split 512+256 for PSUM banks); the two nodes
    of a pair occupy PE quadrants (0,0)/(64,64) via the block-diag weights
  - DVE + ACT evict PSUM->bf16 SBUF with per-partition bias add
  - one [128, GRP*768/2] f32 DMA out (bf16 values packed)
"""
import sys

sys.path.insert(0, "/opt/trn_rl_repo")

import numpy as np

import concourse.bass as bass
import concourse.tile as tile
from concourse import bacc, mybir

F32 = mybir.dt.float32
BF16 = mybir.dt.bfloat16
F8 = mybir.dt.float8e3  # e3m4: 4 mantissa bits, |x|max 15.5 covers randn
N_CORES = 8
B, T, N, C = 32, 24, 1024, 64
K = 3
BT = B * T  # 768
NODES_PER_CORE = N // N_CORES  # 128
PAIRS = NODES_PER_CORE // 2  # 64
GRP = 8  # node pairs per DMA batch ([128, 8*768] bf16 = 1.5 MB transfers)
NGRP = PAIRS // GRP
BT0, BT1 = 512, 256  # psum bank split of BT


def _build_nc(reps: int = 1):
    nc = bacc.Bacc("TRN2", target_bir_lowering=False, debug=False)

    # x holds fp8e3 bytes packed 4-per-f32-word: DMA is element-rate-limited
    # (~100G elem/s), so moving 4 bytes per element quadruples throughput;
    # the matmul rhs bitcasts the tile back to fp8.
    xsh = nc.dram_tensor("xsh", [128, PAIRS, BT // 4], F32, kind="ExternalInput")
    # compact weights: row p holds node (2g + (p>=64))'s weight row p%64
    wsh = nc.dram_tensor("wsh", [128, PAIRS, C], BF16, kind="ExternalInput")
    bcol = nc.dram_tensor("bcol", [128, 1], F32, kind="ExternalInput")
    # output bf16 values, DMA'd as packed f32 words (same element-rate trick)
    osh = nc.dram_tensor("osh", [128, PAIRS, BT // 2], F32, kind="ExternalOutput")

    with tile.TileContext(nc) as tc:
        def body():
            with (
                tc.tile_pool(name="consts", bufs=1) as consts,
                tc.tile_pool(name="xin", bufs=4) as xin,
                tc.tile_pool(name="oout", bufs=3) as oout,
                tc.tile_pool(name="psum", bufs=4, space="PSUM") as psum,
            ):
                wc_sb = consts.tile([128, PAIRS, C], BF16)
                nc.gpsimd.dma_start(wc_sb[:], wsh[:])
                biascol = consts.tile([128, 1], F32)
                nc.gpsimd.dma_start(biascol[:], bcol[:])
                # expand compact W into block-diag pair stationaries on-chip
                w_sbuf = consts.tile([128, PAIRS, 128], BF16)
                nc.gpsimd.memset(w_sbuf[0:C, :, C:128], 0.0)
                nc.vector.memset(w_sbuf[C:128, :, 0:C], 0.0)
                nc.vector.tensor_copy(w_sbuf[0:C, :, 0:C], wc_sb[0:C])
                nc.scalar.copy(w_sbuf[C:128, :, C:128], wc_sb[C:128])

                for sg in range(NGRP):
                    gs = slice(sg * GRP, (sg + 1) * GRP)
                    xt = xin.tile([128, GRP, BT // 4], F32)
                    if sg % 2 == 0:
                        nc.sync.dma_start(xt[:], xsh[:, gs, :])
                    else:
                        nc.scalar.dma_start(xt[:], xsh[:, gs, :])
                    ot = oout.tile([128, GRP, BT], BF16)
                    for j in range(GRP):
                        g = sg * GRP + j
                        ps0 = psum.tile([128, BT0], F32, tag="ps0")
                        ps1 = psum.tile([128, BT1], F32, tag="ps1")
                        w_g = w_sbuf[:, g, :]
                        xf = xt[:, j, :].bitcast(F8)  # [128, BT]
                        nc.tensor.matmul(
                            ps0[:], w_g, xf[:, 0:BT0],
                            start=True, stop=True,
                        )
                        nc.tensor.matmul(
                            ps1[:], w_g, xf[:, BT0:BT],
                            start=True, stop=True,
                        )
                        nc.vector.tensor_scalar_add(
                            ot[:, j, 0:BT0], ps0[:], biascol[:]
                        )
                        nc.scalar.activation(
                            ot[:, j, BT0:BT],
                            ps1[:],
                            mybir.ActivationFunctionType.Identity,
                            bias=biascol[:],
                        )
                    # out: ~71us of bf16 queue time over 16 groups; give the
                    # gpsimd ring (which only carries W otherwise) half, and
                    # the two HWDGE rings (busy with x-in) a quarter each
                    of = ot[:].bitcast(F32)  # [128, GRP, BT//2]
                    if sg % 2 == 1:
                        nc.gpsimd.dma_start(osh[:, gs, :], of)
                    elif sg % 4 == 0:
                        nc.sync.dma_start(osh[:, gs, :], of)
                    else:
                        nc.scalar.dma_start(osh[:, gs, :], of)

        if reps == 1:
            body()
        else:
            with tc.For_i(
                0, reps, 1,
                hint_engines=(
                    mybir.EngineType.PE,
                    mybir.EngineType.Activation,
                    mybir.EngineType.SP,
                    mybir.EngineType.DVE,
                    mybir.EngineType.Pool,
                ),
            ):
                body()

    nc.compile()
    return nc


_RUNNERS: dict = {}


def _get_runner(reps: int = 1):
    key = reps
    if key not in _RUNNERS:
        from runner_inline import build_runner

        nc = _build_nc(reps)
        _RUNNERS[key] = build_runner(nc, N_CORES)
    return _RUNNERS[key]


def _prep_in_maps(x, Tks, Theta, bias):
    import ml_dtypes

    bf16 = ml_dtypes.bfloat16

    x = np.asarray(x, dtype=np.float32)
    Tks = np.asarray(Tks, dtype=np.float32)
    Theta = np.asarray(Theta, dtype=np.float32)
    bias = np.asarray(bias, dtype=np.float32)

    # x is stored as 2x (pushes the e3m4 subnormal boundary down a binade);
    # compensate with W/2. Power of 2, so exact in bf16.
    d = np.ascontiguousarray(np.diagonal(Tks, axis1=1, axis2=2))  # [K, N]
    W = (np.einsum("kn,kco->nco", d, Theta) * 0.5).astype(bf16)  # [N, C, C]
    bias_sum = bias.sum(axis=0)  # [C]
    bcol = np.ascontiguousarray(
        np.concatenate([bias_sum, bias_sum]).reshape(128, 1)
    ).astype(np.float32)

    xr = x.reshape(BT, N, C).transpose(1, 2, 0)  # [N, C, BT] view
    in_maps = []
    for i in range(N_CORES):
        lo = i * NODES_PER_CORE
        slab = np.ascontiguousarray(
            xr[lo : lo + NODES_PER_CORE]
        )  # [128, C, BT]
        # [pair, half, c, bt] -> [(half, c), pair, bt]
        xcore = (
            np.ascontiguousarray(
                slab.reshape(PAIRS, 2, C, BT).transpose(1, 2, 0, 3)
            ).reshape(128, PAIRS, BT)
            * 2.0
        ).astype(ml_dtypes.float8_e3m4).view(np.float32)  # packed 4/word

        Wc = W[lo : lo + NODES_PER_CORE]  # [128, C, C] bf16
        wblk = np.empty((128, PAIRS, C), dtype=bf16)
        wblk[0:C] = Wc[0::2].transpose(1, 0, 2)  # [c, g, o]
        wblk[C:128] = Wc[1::2].transpose(1, 0, 2)

        in_maps.append(
            {"xsh": xcore, "wsh": wblk, "bcol": bcol}
        )
    return in_maps


def _gather(results):
    import ml_dtypes

    slabs = []
    for r in results:
        o = (
            np.ascontiguousarray(np.asarray(r["osh"]))  # [128, PAIRS, BT//2] f32
            .view(ml_dtypes.bfloat16)
            .astype(np.float32)
        )  # [128, PAIRS, BT]
        # [(half, o), pair, bt] -> [(pair, half), o, bt]
        o = o.reshape(2, C, PAIRS, BT).transpose(2, 0, 1, 3)
        slabs.append(o.reshape(NODES_PER_CORE, C, BT))
    full = np.concatenate(slabs, axis=0)  # [N, C_OUT, BT]
    return np.ascontiguousarray(full.transpose(2, 0, 1)).reshape(B, T, N, C)


def kernel(x, Tks, Theta, bias):
    run = _get_runner(reps=1)
    in_maps = _prep_in_maps(x, Tks, Theta, bias)
    results, _ = run(in_maps)
    return _gather(results)


# ---------------------------------------------------------------------------
# Inline PJRT SPMD runner (kernel.py must be self-contained).
# ---------------------------------------------------------------------------
import importlib.util as _ilu
import types as _types

_runner_src = '''
import time
import numpy as np
import jax
from jax.sharding import Mesh, PartitionSpec
from jax.experimental.shard_map import shard_map

from concourse import mybir
from concourse.bass2jax import _bass_exec_p, install_neuronx_cc_hook, partition_id_tensor


def build_runner(nc, n_cores):
    install_neuronx_cc_hook()

    partition_name = nc.partition_id_tensor.name if nc.partition_id_tensor else None

    in_names, out_names, out_avals, zero_shapes = [], [], [], []
    for alloc in nc.m.functions[0].allocations:
        if not isinstance(alloc, mybir.MemoryLocationSet):
            continue
        name = alloc.memorylocations[0].name
        if alloc.kind == "ExternalInput":
            if name != partition_name:
                in_names.append(name)
        elif alloc.kind == "ExternalOutput":
            shape = tuple(alloc.tensor_shape)
            dtype = mybir.dt.np(alloc.dtype)
            out_names.append(name)
            out_avals.append(jax.core.ShapedArray(shape, dtype))
            zero_shapes.append((shape, dtype))

    n_params = len(in_names)
    n_outs = len(out_names)
    all_in_names = list(in_names) + list(out_names)
    if partition_name is not None:
        all_in_names.append(partition_name)
    donate = tuple(range(n_params, n_params + n_outs))

    def _body(*args):
        operands = list(args)
        if partition_name is not None:
            operands.append(partition_id_tensor())
        outs = _bass_exec_p.bind(
            *operands,
            out_avals=tuple(out_avals),
            in_names=tuple(all_in_names),
            out_names=tuple(out_names),
            lowering_input_output_aliases=(),
            sim_require_finite=True,
            sim_require_nnan=True,
            nc=nc,
        )
        return tuple(outs)

    devices = jax.devices()[:n_cores]
    mesh = Mesh(np.asarray(devices), ("core",))
    in_specs = (PartitionSpec("core"),) * (n_params + n_outs)
    out_specs = (PartitionSpec("core"),) * n_outs
    sharded = jax.jit(
        shard_map(_body, mesh=mesh, in_specs=in_specs, out_specs=out_specs,
                  check_rep=False),
        donate_argnums=donate,
        keep_unused=True,
    )

    def run(in_maps, time_iters=0):
        per_core = [[np.asarray(m[name]) for name in in_names] for m in in_maps]
        concat_in = [
            np.concatenate([per_core[c][i] for c in range(n_cores)], axis=0)
            for i in range(n_params)
        ]
        in_dev = [jax.device_put(a) for a in concat_in]
        jax.block_until_ready(in_dev)

        def zeros_dev():
            z = [
                jax.device_put(np.zeros((n_cores * s[0], *s[1:]), d))
                for (s, d) in zero_shapes
            ]
            jax.block_until_ready(z)
            return z

        out_arrs = sharded(*in_dev, *zeros_dev())
        jax.block_until_ready(out_arrs)

        times = []
        for _ in range(time_iters):
            z = zeros_dev()
            t0 = time.perf_counter()
            out2 = sharded(*in_dev, *z)
            jax.block_until_ready(out2)
            times.append(time.perf_counter() - t0)
            del out2

        results = [
            {
                name: np.asarray(out_arrs[i]).reshape(n_cores, *out_avals[i].shape)[c]
                for i, name in enumerate(out_names)
            }
            for c in range(n_cores)
        ]
        return results, times

    return run
'''

_mod = _types.ModuleType("runner_inline")
exec(compile(_runner_src, "runner_inline", "exec"), _mod.__dict__)
sys.modules["runner_inline"] = _mod


# revision 32
# speedup vs baseline: 1.5493x; 1.5493x over previous
"""Trainium2 Bass kernel for ChebGraphConv forward.

Reference math:
    d = diagonal(Tks, axis1=1, axis2=2)                  # [K, N]
    out = einsum('kn,btnc,kco->btno', d, x, Theta) + sum_k bias[k]

Reformulation: per-node weight W_n = sum_k d[k,n] * Theta[k]  (64x64),
then out[bt, n, :] = x[bt, n, :] @ W_n + bias_sum.

The kernel is HBM/DMA-bound (x in + out dominate). Precision plan within
the 2e-2 rel-err budget: x is quantized host-side to fp8 e3m4 (stored as
2x to keep small values out of the subnormal range; W carries the /2),
W is bf16 (compact, expanded to block-diag on-chip), PSUM accumulates
f32, output is written bf16 and upcast on host. End-to-end rel-err
~1.35e-2 (x-quantization dominates).

DMA on this part is element-rate-limited (~85G elem/s per DGE ring,
byte width free), so all bulk transfers ride as packed f32 words:
x fp8 bytes 4-per-word (bitcast back to fp8 for the matmul rhs), out
bf16 2-per-word. Transfers are split across the three rings (SP /
Activation HWDGE + gpsimd SWDGE) to stay under per-ring caps.

Distribution: shard the N=1024 nodes over 8 cores (128 nodes each); every
core sees all BT=768 (batch*time) rows. Host pre-permutes x to
[p=(pair_half, c), pair, bt] so every DMA is a plain 2D transfer with
KB-scale contiguous runs per partition.

Per group of GRP node pairs:
  - one [128, GRP*768/4] f32 DMA in (fp8 bytes packed)
  - per pair: 2 matmuls (bt split 512+256 for PSUM banks); the two nodes
    of a pair occupy PE quadrants (0,0)/(64,64) via the block-diag weights
  - DVE + ACT evict PSUM->bf16 SBUF with per-partition bias add
  - one [128, GRP*768/2] f32 DMA out (bf16 values packed)
"""
import sys

sys.path.insert(0, "/opt/trn_rl_repo")

import numpy as np

import concourse.bass as bass
import concourse.tile as tile
from concourse import bacc, mybir

F32 = mybir.dt.float32
BF16 = mybir.dt.bfloat16
F8 = mybir.dt.float8e3  # e3m4: 4 mantissa bits, |x|max 15.5 covers randn
N_CORES = 8
B, T, N, C = 32, 24, 1024, 64
K = 3
BT = B * T  # 768
NODES_PER_CORE = N // N_CORES  # 128
PAIRS = NODES_PER_CORE // 2  # 64
GRP = 8  # node pairs per DMA batch ([128, 8*768] bf16 = 1.5 MB transfers)
NGRP = PAIRS // GRP
BT0, BT1 = 512, 256  # psum bank split of BT


def _build_nc(reps: int = 1):
    nc = bacc.Bacc("TRN2", target_bir_lowering=False, debug=False)

    # x holds fp8e3 bytes packed 4-per-f32-word: DMA is element-rate-limited
    # (~100G elem/s), so moving 4 bytes per element quadruples throughput;
    # the matmul rhs bitcasts the tile back to fp8.
    xsh = nc.dram_tensor("xsh", [128, PAIRS, BT // 4], F32, kind="ExternalInput")
    # compact weights: row p holds node (2g + (p>=64))'s weight row p%64
    wsh = nc.dram_tensor("wsh", [128, PAIRS, C], BF16, kind="ExternalInput")
    bcol = nc.dram_tensor("bcol", [128, 1], F32, kind="ExternalInput")
    osh = nc.dram_tensor("osh", [128, PAIRS, BT], BF16, kind="ExternalOutput")

    with tile.TileContext(nc) as tc:
        def body():
            with (
                tc.tile_pool(name="consts", bufs=1) as consts,
                tc.tile_pool(name="xin", bufs=4) as xin,
                tc.tile_pool(name="oout", bufs=3) as oout,
                tc.tile_pool(name="psum", bufs=4, space="PSUM") as psum,
            ):
                wc_sb = consts.tile([128, PAIRS, C], BF16)
                nc.gpsimd.dma_start(wc_sb[:], wsh[:])
                biascol = consts.tile([128, 1], F32)
                nc.gpsimd.dma_start(biascol[:], bcol[:])
                # expand compact W into block-diag pair stationaries on-chip
                w_sbuf = consts.tile([128, PAIRS, 128], BF16)
                nc.gpsimd.memset(w_sbuf[0:C, :, C:128], 0.0)
                nc.vector.memset(w_sbuf[C:128, :, 0:C], 0.0)
                nc.vector.tensor_copy(w_sbuf[0:C, :, 0:C], wc_sb[0:C])
                nc.scalar.copy(w_sbuf[C:128, :, C:128], wc_sb[C:128])

                for sg in range(NGRP):
                    gs = slice(sg * GRP, (sg + 1) * GRP)
                    xt = xin.tile([128, GRP, BT // 4], F32)
                    if sg % 2 == 0:
                        nc.sync.dma_start(xt[:], xsh[:, gs, :])
                    else:
                        nc.scalar.dma_start(xt[:], xsh[:, gs, :])
                    ot = oout.tile([128, GRP, BT], BF16)
                    for j in range(GRP):
                        g = sg * GRP + j
                        ps0 = psum.tile([128, BT0], F32, tag="ps0")
                        ps1 = psum.tile([128, BT1], F32, tag="ps1")
                        w_g = w_sbuf[:, g, :]
                        xf = xt[:, j, :].bitcast(F8)  # [128, BT]
                        nc.tensor.matmul(
                            ps0[:], w_g, xf[:, 0:BT0],
                            start=True, stop=True,
                        )
                        nc.tensor.matmul(
                            ps1[:], w_g, xf[:, BT0:BT],
                            start=True, stop=True,
                        )
                        nc.vector.tensor_scalar_add(
                            ot[:, j, 0:BT0], ps0[:], biascol[:]
                        )
                        nc.scalar.activation(
                            ot[:, j, BT0:BT],
                            ps1[:],
                            mybir.ActivationFunctionType.Identity,
                            bias=biascol[:],
                        )
                    # out: ~71us of bf16 queue time over 16 groups; give the
                    # gpsimd ring (which only carries W otherwise) half, and
                    # the two HWDGE rings (busy with x-in) a quarter each
                    if sg % 2 == 1:
                        nc.gpsimd.dma_start(osh[:, gs, :], ot[:])
                    elif sg % 4 == 0:
                        nc.sync.dma_start(osh[:, gs, :], ot[:])
                    else:
                        nc.scalar.dma_start(osh[:, gs, :], ot[:])

        if reps == 1:
            body()
        else:
            with tc.For_i(
                0, reps, 1,
                hint_engines=(
                    mybir.EngineType.PE,
                    mybir.EngineType.Activation,
                    mybir.EngineType.SP,
                    mybir.EngineType.DVE,
                    mybir.EngineType.Pool,
                ),
            ):
                body()

    nc.compile()
    return nc


_RUNNERS: dict = {}


def _get_runner(reps: int = 1):
    key = reps
    if key not in _RUNNERS:
        from runner_inline import build_runner

        nc = _build_nc(reps)
        _RUNNERS[key] = build_runner(nc, N_CORES)
    return _RUNNERS[key]


def _prep_in_maps(x, Tks, Theta, bias):
    import ml_dtypes

    bf16 = ml_dtypes.bfloat16

    x = np.asarray(x, dtype=np.float32)
    Tks = np.asarray(Tks, dtype=np.float32)
    Theta = np.asarray(Theta, dtype=np.float32)
    bias = np.asarray(bias, dtype=np.float32)

    # x is stored as 2x (pushes the e3m4 subnormal boundary down a binade);
    # compensate with W/2. Power of 2, so exact in bf16.
    d = np.ascontiguousarray(np.diagonal(Tks, axis1=1, axis2=2))  # [K, N]
    W = (np.einsum("kn,kco->nco", d, Theta) * 0.5).astype(bf16)  # [N, C, C]
    bias_sum = bias.sum(axis=0)  # [C]
    bcol = np.ascontiguousarray(
        np.concatenate([bias_sum, bias_sum]).reshape(128, 1)
    ).astype(np.float32)

    xr = x.reshape(BT, N, C).transpose(1, 2, 0)  # [N, C, BT] view
    in_maps = []
    for i in range(N_CORES):
        lo = i * NODES_PER_CORE
        slab = np.ascontiguousarray(
            xr[lo : lo + NODES_PER_CORE]
        )  # [128, C, BT]
        # [pair, half, c, bt] -> [(half, c), pair, bt]
        xcore = (
            np.ascontiguousarray(
                slab.reshape(PAIRS, 2, C, BT).transpose(1, 2, 0, 3)
            ).reshape(128, PAIRS, BT)
            * 2.0
        ).astype(ml_dtypes.float8_e3m4).view(np.float32)  # packed 4/word

        Wc = W[lo : lo + NODES_PER_CORE]  # [128, C, C] bf16
        wblk = np.empty((128, PAIRS, C), dtype=bf16)
        wblk[0:C] = Wc[0::2].transpose(1, 0, 2)  # [c, g, o]
        wblk[C:128] = Wc[1::2].transpose(1, 0, 2)

        in_maps.append(
            {"xsh": xcore, "wsh": wblk, "bcol": bcol}
        )
    return in_maps


def _gather(results):
    import ml_dtypes

    slabs = []
    for r in results:
        o = np.asarray(r["osh"]).astype(np.float32)  # [128, PAIRS, BT]
        # [(half, o), pair, bt] -> [(pair, half), o, bt]
        o = o.reshape(2, C, PAIRS, BT).transpose(2, 0, 1, 3)
        slabs.append(o.reshape(NODES_PER_CORE, C, BT))
    full = np.concatenate(slabs, axis=0)  # [N, C_OUT, BT]
    return np.ascontiguousarray(full.transpose(2, 0, 1)).reshape(B, T, N, C)


def kernel(x, Tks, Theta, bias):
    run = _get_runner(reps=1)
    in_maps = _prep_in_maps(x, Tks, Theta, bias)
    results, _ = run(in_maps)
    return _gather(results)


# ---------------------------------------------------------------------------
# Inline PJRT SPMD runner (kernel.py must be self-contained).
# ---------------------------------------------------------------------------
import importlib.util as _ilu
import types as _types

_runner_src = '''
import time
import numpy as np
import jax
from jax.sharding import Mesh, PartitionSpec
from jax.experimental.shard_map import shard_map

from concourse import mybir
from concourse.bass2jax import _bass_exec_p, install_neuronx_cc_hook, partition_id_tensor


def build_runner(nc, n_cores):
    install_neuronx_cc_hook()

    partition_name = nc.partition_id_tensor.name if nc.partition_id_tensor else None

    in_names, out_names, out_avals, zero_shapes = [], [], [], []
    for alloc in nc.m.functions[0].allocations:
        if not isinstance(alloc, mybir.MemoryLocationSet):
            continue
        name = alloc.memorylocations[0].name
        if alloc.kind == "ExternalInput":
            if name != partition_name:
                in_names.append(name)
        elif alloc.kind == "ExternalOutput":
            shape = tuple(alloc.tensor_shape)
            dtype = mybir.dt.np(alloc.dtype)
            out_names.append(name)
            out_avals.append(jax.core.ShapedArray(shape, dtype))
            zero_shapes.append((shape, dtype))

    n_params = len(in_names)
    n_outs = len(out_names)
    all_in_names = list(in_names) + list(out_names)
    if partition_name is not None:
        all_in_names.append(partition_name)
    donate = tuple(range(n_params, n_params + n_outs))

    def _body(*args):
        operands = list(args)
        if partition_name is not None:
            operands.append(partition_id_tensor())
        outs = _bass_exec_p.bind(
            *operands,
            out_avals=tuple(out_avals),
            in_names=tuple(all_in_names),
            out_names=tuple(out_names),
            lowering_input_output_aliases=(),
            sim_require_finite=True,
            sim_require_nnan=True,
            nc=nc,
        )
        return tuple(outs)

    devices = jax.devices()[:n_cores]
    mesh = Mesh(np.asarray(devices), ("core",))
    in_specs = (PartitionSpec("core"),) * (n_params + n_outs)
    out_specs = (PartitionSpec("core"),) * n_outs
    sharded = jax.jit(
        shard_map(_body, mesh=mesh, in_specs=in_specs, out_specs=out_specs,
                  check_rep=False),
        donate_argnums=donate,
        keep_unused=True,
    )

    def run(in_maps, time_iters=0):
        per_core = [[np.asarray(m[name]) for name in in_names] for m in in_maps]
        concat_in = [
            np.concatenate([per_core[c][i] for c in range(n_cores)], axis=0)
            for i in range(n_params)
        ]
        in_dev = [jax.device_put(a) for a in concat_in]
        jax.block_until_ready(in_dev)

        def zeros_dev():
            z = [
                jax.device_put(np.zeros((n_cores * s[0], *s[1:]), d))
                for (s, d) in zero_shapes
            ]
            jax.block_until_ready(z)
            return z

        out_arrs = sharded(*in_dev, *zeros_dev())
        jax.block_until_ready(out_arrs)

        times = []
        for _ in range(time_iters):
            z = zeros_dev()
            t0 = time.perf_counter()
            out2 = sharded(*in_dev, *z)
            jax.block_until_ready(out2)
            times.append(time.perf_counter() - t0)
            del out2

        results = [
            {
                name: np.asarray(out_arrs[i]).reshape(n_cores, *out_avals[i].shape)[c]
                for i, name in enumerate(out_names)
            }
            for c in range(n_cores)
        ]
        return results, times

    return run
'''

_mod = _types.ModuleType("runner_inline")
exec(compile(_runner_src, "runner_inline", "exec"), _mod.__dict__)
sys.modules["runner_inline"] = _mod


# revision 33
# speedup vs baseline: 1.7097x; 1.1036x over previous
"""Trainium2 Bass kernel for ChebGraphConv forward.

Reference math:
    d = diagonal(Tks, axis1=1, axis2=2)                  # [K, N]
    out = einsum('kn,btnc,kco->btno', d, x, Theta) + sum_k bias[k]

Reformulation: per-node weight W_n = sum_k d[k,n] * Theta[k]  (64x64),
then out[bt, n, :] = x[bt, n, :] @ W_n + bias_sum.

The kernel is HBM/DMA-bound (x in + out dominate). Precision plan within
the 2e-2 rel-err budget: x is quantized host-side to fp8 e3m4 (stored as
2x to keep small values out of the subnormal range; W carries the /2),
W is bf16 (compact, expanded to block-diag on-chip), PSUM accumulates
f32, output is written bf16 and upcast on host. End-to-end rel-err
~1.35e-2 (x-quantization dominates).

DMA on this part is element-rate-limited (~85G elem/s per DGE ring,
byte width free), so all bulk transfers ride as packed f32 words:
x fp8 bytes 4-per-word (bitcast back to fp8 for the matmul rhs), out
bf16 2-per-word. Transfers are split across the three rings (SP /
Activation HWDGE + gpsimd SWDGE) to stay under per-ring caps.

Distribution: shard the N=1024 nodes over 8 cores (128 nodes each); every
core sees all BT=768 (batch*time) rows. Host pre-permutes x to
[p=(pair_half, c), pair, bt] so every DMA is a plain 2D transfer with
KB-scale contiguous runs per partition.

Per group of GRP node pairs:
  - one [128, GRP*768/4] f32 DMA in (fp8 bytes packed)
  - per pair: 2 matmuls (bt split 512+256 for PSUM banks); the two nodes
    of a pair occupy PE quadrants (0,0)/(64,64) via the block-diag weights
  - DVE + ACT evict PSUM->bf16 SBUF with per-partition bias add
  - one [128, GRP*768/2] f32 DMA out (bf16 values packed)
"""
import sys

sys.path.insert(0, "/opt/trn_rl_repo")

import numpy as np

import concourse.bass as bass
import concourse.tile as tile
from concourse import bacc, mybir

F32 = mybir.dt.float32
BF16 = mybir.dt.bfloat16
F8 = mybir.dt.float8e3  # e3m4: 4 mantissa bits, |x|max 15.5 covers randn
N_CORES = 8
B, T, N, C = 32, 24, 1024, 64
K = 3
BT = B * T  # 768
NODES_PER_CORE = N // N_CORES  # 128
PAIRS = NODES_PER_CORE // 2  # 64
GRP = 8  # node pairs per DMA batch ([128, 8*768] bf16 = 1.5 MB transfers)
NGRP = PAIRS // GRP
BT0, BT1 = 512, 256  # psum bank split of BT


def _build_nc(reps: int = 1):
    nc = bacc.Bacc("TRN2", target_bir_lowering=False, debug=False)

    # x holds fp8e3 bytes packed 4-per-f32-word: DMA is element-rate-limited
    # (~100G elem/s), so moving 4 bytes per element quadruples throughput;
    # the matmul rhs bitcasts the tile back to fp8.
    xsh = nc.dram_tensor("xsh", [128, PAIRS, BT // 4], F32, kind="ExternalInput")
    # compact weights: row p holds node (2g + (p>=64))'s weight row p%64
    wsh = nc.dram_tensor("wsh", [128, PAIRS, C], BF16, kind="ExternalInput")
    bcol = nc.dram_tensor("bcol", [128, 1], F32, kind="ExternalInput")
    osh = nc.dram_tensor("osh", [128, PAIRS, BT], BF16, kind="ExternalOutput")

    with tile.TileContext(nc) as tc:
        def trace(loop):
            with (
                tc.tile_pool(name="consts", bufs=1) as consts,
                tc.tile_pool(name="wcp", bufs=2) as wcpool,
                tc.tile_pool(name="xin", bufs=6) as xin,
                tc.tile_pool(name="oout", bufs=4) as oout,
                tc.tile_pool(name="psum", bufs=4, space="PSUM") as psum,
            ):
                biascol = consts.tile([128, 1], F32, tag="bias")
                wA = consts.tile([128, PAIRS, 128], BF16, tag="wA")
                wB = consts.tile([128, PAIRS, 128], BF16, tag="wB")

                def load_w(w_dst, ring):
                    # compact -> block-diag expand on DVE/ACT (both have
                    # slack); prefetched so it stays off the PE critical path
                    wc_sb = wcpool.tile([128, PAIRS, C], BF16, tag="wc")
                    ring.dma_start(wc_sb[:], wsh[:])
                    nc.gpsimd.memset(w_dst[0:C, :, C:128], 0.0)
                    nc.vector.memset(w_dst[C:128, :, 0:C], 0.0)
                    nc.vector.tensor_copy(w_dst[0:C, :, 0:C], wc_sb[0:C])
                    nc.scalar.copy(w_dst[C:128, :, C:128], wc_sb[C:128])

                def half(w_cur, prep):
                    for sg in range(NGRP):
                        if sg == NGRP // 2 and prep is not None:
                            prep()  # prefetch next half's W mid-half
                        gs = slice(sg * GRP, (sg + 1) * GRP)
                        xt = xin.tile([128, GRP, BT // 4], F32)
                        if sg % 2 == 0:
                            nc.sync.dma_start(xt[:], xsh[:, gs, :])
                        else:
                            nc.scalar.dma_start(xt[:], xsh[:, gs, :])
                        ot = oout.tile([128, GRP, BT], BF16)
                        for j in range(GRP):
                            g = sg * GRP + j
                            ps0 = psum.tile([128, BT0], F32, tag="ps0")
                            ps1 = psum.tile([128, BT1], F32, tag="ps1")
                            w_g = w_cur[:, g, :]
                            xf = xt[:, j, :].bitcast(F8)  # [128, BT]
                            nc.tensor.matmul(
                                ps0[:], w_g, xf[:, 0:BT0],
                                start=True, stop=True,
                            )
                            nc.tensor.matmul(
                                ps1[:], w_g, xf[:, BT0:BT],
                                start=True, stop=True,
                            )
                            nc.vector.tensor_scalar_add(
                                ot[:, j, 0:BT0], ps0[:], biascol[:]
                            )
                            nc.scalar.activation(
                                ot[:, j, BT0:BT],
                                ps1[:],
                                mybir.ActivationFunctionType.Identity,
                                bias=biascol[:],
                            )
                        # out: gpsimd ring (otherwise idle) takes half, the
                        # two HWDGE rings (also carrying x-in) a quarter each
                        if sg % 2 == 1:
                            nc.gpsimd.dma_start(osh[:, gs, :], ot[:])
                        elif sg % 4 == 0:
                            nc.sync.dma_start(osh[:, gs, :], ot[:])
                        else:
                            nc.scalar.dma_start(osh[:, gs, :], ot[:])

                def body(two_reps):
                    # one For_i iteration = two reps with double-buffered W:
                    # half A prefetches wB mid-flight, so only wA's load
                    # (first on the lightly-loaded SP ring) can ever stall
                    # a rep boundary - once per two reps
                    nc.gpsimd.dma_start(biascol[:], bcol[:])
                    load_w(wA, nc.sync)
                    if two_reps:
                        half(wA, lambda: load_w(wB, nc.gpsimd))
                        half(wB, None)
                    else:
                        half(wA, None)

                loop(body)

        if reps == 1:
            trace(lambda body: body(False))
        else:
            assert reps % 2 == 0, "timing reps must be even"

            def loop(body):
                with tc.For_i(
                    0, reps // 2, 1,
                    hint_engines=(
                        mybir.EngineType.PE,
                        mybir.EngineType.Activation,
                        mybir.EngineType.SP,
                        mybir.EngineType.DVE,
                        mybir.EngineType.Pool,
                    ),
                ):
                    body(True)

            trace(loop)

    nc.compile()
    return nc


_RUNNERS: dict = {}


def _get_runner(reps: int = 1):
    key = reps
    if key not in _RUNNERS:
        from runner_inline import build_runner

        nc = _build_nc(reps)
        _RUNNERS[key] = build_runner(nc, N_CORES)
    return _RUNNERS[key]


def _prep_in_maps(x, Tks, Theta, bias):
    import ml_dtypes

    bf16 = ml_dtypes.bfloat16

    x = np.asarray(x, dtype=np.float32)
    Tks = np.asarray(Tks, dtype=np.float32)
    Theta = np.asarray(Theta, dtype=np.float32)
    bias = np.asarray(bias, dtype=np.float32)

    # x is stored as 2x (pushes the e3m4 subnormal boundary down a binade);
    # compensate with W/2. Power of 2, so exact in bf16.
    d = np.ascontiguousarray(np.diagonal(Tks, axis1=1, axis2=2))  # [K, N]
    W = (np.einsum("kn,kco->nco", d, Theta) * 0.5).astype(bf16)  # [N, C, C]
    bias_sum = bias.sum(axis=0)  # [C]
    bcol = np.ascontiguousarray(
        np.concatenate([bias_sum, bias_sum]).reshape(128, 1)
    ).astype(np.float32)

    xr = x.reshape(BT, N, C).transpose(1, 2, 0)  # [N, C, BT] view
    in_maps = []
    for i in range(N_CORES):
        lo = i * NODES_PER_CORE
        slab = np.ascontiguousarray(
            xr[lo : lo + NODES_PER_CORE]
        )  # [128, C, BT]
        # [pair, half, c, bt] -> [(half, c), pair, bt]
        xcore = (
            np.ascontiguousarray(
                slab.reshape(PAIRS, 2, C, BT).transpose(1, 2, 0, 3)
            ).reshape(128, PAIRS, BT)
            * 2.0
        ).astype(ml_dtypes.float8_e3m4).view(np.float32)  # packed 4/word

        Wc = W[lo : lo + NODES_PER_CORE]  # [128, C, C] bf16
        wblk = np.empty((128, PAIRS, C), dtype=bf16)
        wblk[0:C] = Wc[0::2].transpose(1, 0, 2)  # [c, g, o]
        wblk[C:128] = Wc[1::2].transpose(1, 0, 2)

        in_maps.append(
            {"xsh": xcore, "wsh": wblk, "bcol": bcol}
        )
    return in_maps


def _gather(results):
    import ml_dtypes

    slabs = []
    for r in results:
        o = np.asarray(r["osh"]).astype(np.float32)  # [128, PAIRS, BT]
        # [(half, o), pair, bt] -> [(pair, half), o, bt]
        o = o.reshape(2, C, PAIRS, BT).transpose(2, 0, 1, 3)
        slabs.append(o.reshape(NODES_PER_CORE, C, BT))
    full = np.concatenate(slabs, axis=0)  # [N, C_OUT, BT]
    return np.ascontiguousarray(full.transpose(2, 0, 1)).reshape(B, T, N, C)


def kernel(x, Tks, Theta, bias):
    run = _get_runner(reps=1)
    in_maps = _prep_in_maps(x, Tks, Theta, bias)
    results, _ = run(in_maps)
    return _gather(results)


# ---------------------------------------------------------------------------
# Inline PJRT SPMD runner (kernel.py must be self-contained).
# ---------------------------------------------------------------------------
import importlib.util as _ilu
import types as _types

_runner_src = '''
import time
import numpy as np
import jax
from jax.sharding import Mesh, PartitionSpec
from jax.experimental.shard_map import shard_map

from concourse import mybir
from concourse.bass2jax import _bass_exec_p, install_neuronx_cc_hook, partition_id_tensor


def build_runner(nc, n_cores):
    install_neuronx_cc_hook()

    partition_name = nc.partition_id_tensor.name if nc.partition_id_tensor else None

    in_names, out_names, out_avals, zero_shapes = [], [], [], []
    for alloc in nc.m.functions[0].allocations:
        if not isinstance(alloc, mybir.MemoryLocationSet):
            continue
        name = alloc.memorylocations[0].name
        if alloc.kind == "ExternalInput":
            if name != partition_name:
                in_names.append(name)
        elif alloc.kind == "ExternalOutput":
            shape = tuple(alloc.tensor_shape)
            dtype = mybir.dt.np(alloc.dtype)
            out_names.append(name)
            out_avals.append(jax.core.ShapedArray(shape, dtype))
            zero_shapes.append((shape, dtype))

    n_params = len(in_names)
    n_outs = len(out_names)
    all_in_names = list(in_names) + list(out_names)
    if partition_name is not None:
        all_in_names.append(partition_name)
    donate = tuple(range(n_params, n_params + n_outs))

    def _body(*args):
        operands = list(args)
        if partition_name is not None:
            operands.append(partition_id_tensor())
        outs = _bass_exec_p.bind(
            *operands,
            out_avals=tuple(out_avals),
            in_names=tuple(all_in_names),
            out_names=tuple(out_names),
            lowering_input_output_aliases=(),
            sim_require_finite=True,
            sim_require_nnan=True,
            nc=nc,
        )
        return tuple(outs)

    devices = jax.devices()[:n_cores]
    mesh = Mesh(np.asarray(devices), ("core",))
    in_specs = (PartitionSpec("core"),) * (n_params + n_outs)
    out_specs = (PartitionSpec("core"),) * n_outs
    sharded = jax.jit(
        shard_map(_body, mesh=mesh, in_specs=in_specs, out_specs=out_specs,
                  check_rep=False),
        donate_argnums=donate,
        keep_unused=True,
    )

    def run(in_maps, time_iters=0):
        per_core = [[np.asarray(m[name]) for name in in_names] for m in in_maps]
        concat_in = [
            np.concatenate([per_core[c][i] for c in range(n_cores)], axis=0)
            for i in range(n_params)
        ]
        in_dev = [jax.device_put(a) for a in concat_in]
        jax.block_until_ready(in_dev)

        def zeros_dev():
            z = [
                jax.device_put(np.zeros((n_cores * s[0], *s[1:]), d))
                for (s, d) in zero_shapes
            ]
            jax.block_until_ready(z)
            return z

        out_arrs = sharded(*in_dev, *zeros_dev())
        jax.block_until_ready(out_arrs)

        times = []
        for _ in range(time_iters):
            z = zeros_dev()
            t0 = time.perf_counter()
            out2 = sharded(*in_dev, *z)
            jax.block_until_ready(out2)
            times.append(time.perf_counter() - t0)
            del out2

        results = [
            {
                name: np.asarray(out_arrs[i]).reshape(n_cores, *out_avals[i].shape)[c]
                for i, name in enumerate(out_names)
            }
            for c in range(n_cores)
        ]
        return results, times

    return run
'''

_mod = _types.ModuleType("runner_inline")
exec(compile(_runner_src, "runner_inline", "exec"), _mod.__dict__)
sys.modules["runner_inline"] = _mod
